# revision 4
# baseline (speedup 1.0000x reference)
"""Trainium2 Bass kernel for Conv2dWeightModulate (StyleGAN2-style modulated conv).

Full-input contract: kernel(feature_map, style, weight) -> (B, Cout, H, W).
Data-parallel over the batch: each of the 8 NeuronCores handles one sample.

Math (per sample b):
    w_mod[o,i,ky,kx] = gain * style[b,i] * weight[o,i,ky,kx]     (gain = 1/sqrt(Cin*K*K))
    sigma_inv[o]     = rsqrt(sum_{i,ky,kx} w_mod^2 + 1e-8)
    y[b,o]           = sigma_inv[o] * conv2d_same(x[b], w_mod)[o]

On device, the modulated weight lives as 36 SBUF tiles w_r[it,kk] of shape
(i=128, o=512) in float32r (rounded fp32 that runs the PE at full bf16 rate).
The conv is 9 shifted accumulating matmuls per (output-tile, pixel-tile) over a
zero-padded 66x66 image held in SBUF.
"""
from contextlib import ExitStack

import numpy as np

import concourse.bass as bass
import concourse.bacc as bacc
import concourse.mybir as mybir
import concourse.tile as tile
from concourse.bass_utils import run_bass_kernel_spmd

B = 8
CIN = 512
COUT = 512
KS = 3
H = 64
W = 64
HP = H + 2  # padded
WP = W + 2
PIX = H * W
IT = CIN // 128   # input-channel tiles
OT = COUT // 128  # output-channel tiles
NT = 8            # pixel tiles of 512 px = 8 image rows each
ROWS_PER_NT = 512 // W
GAIN = 1.0 / float(np.sqrt(CIN * KS * KS))
EPS = 1e-8

F32 = mybir.dt.float32
F32R = mybir.dt.float32r

_NC_CACHE = None


def _build():
    nc = bacc.Bacc("TRN2", target_bir_lowering=False)
    x_d = nc.dram_tensor("x", [CIN, PIX], F32, kind="ExternalInput")
    s_d = nc.dram_tensor("style4", [IT, 128], F32, kind="ExternalInput")
    w_d = nc.dram_tensor("wt", [KS * KS, CIN, COUT], F32, kind="ExternalInput")
    y_d = nc.dram_tensor("y", [COUT, PIX], F32, kind="ExternalOutput")

    with tile.TileContext(nc) as tc, ExitStack() as ctx:
        persist = ctx.enter_context(tc.tile_pool(name="persist", bufs=1))
        xtmp_p = ctx.enter_context(tc.tile_pool(name="xtmp", bufs=2))
        wtmp_p = ctx.enter_context(tc.tile_pool(name="wtmp", bufs=3))
        wsq_p = ctx.enter_context(tc.tile_pool(name="wsq", bufs=2))
        out_p = ctx.enter_context(tc.tile_pool(name="outp", bufs=4))
        cps_p = ctx.enter_context(tc.tile_pool(name="cpsum", bufs=4, space="PSUM"))
        sps_p = ctx.enter_context(tc.tile_pool(name="spsum", bufs=1, space="PSUM"))

        # ---- style: s_gain[it] = gain * style ----
        s_gain = []
        for it in range(IT):
            sr = persist.tile([128, 1], F32, tag=f"sraw{it}")
            nc.sync.dma_start(
                out=sr,
                in_=s_d[it, :].rearrange("(p one) -> p one", one=1),
            )
            sg = persist.tile([128, 1], F32, tag=f"sg{it}")
            nc.vector.tensor_scalar_mul(sg[:, :], sr[:, :], GAIN)
            s_gain.append(sg)

        ones = persist.tile([128, 1], F32, tag="ones")
        nc.vector.memset(ones[:, :], 1.0)

        # ---- x: zero-padded 66x66 image per input-channel tile, fp32r ----
        # memset can't write f32r; produce halo zeros via a rounded copy.
        zero_f = persist.tile([128, HP], F32, tag="zero_f")
        nc.vector.memset(zero_f[:, :], 0.0)
        zero_r = persist.tile([128, HP], F32R, tag="zero_r")
        nc.vector.tensor_copy(zero_r[:, :], zero_f[:, :])
        xp = []
        for it in range(IT):
            t = persist.tile([128, HP, WP], F32R, tag=f"xp{it}")
            xp.append(t)
            nc.vector.tensor_copy(t[:, 0, :], zero_r[:, :])
            nc.vector.tensor_copy(t[:, HP - 1, :], zero_r[:, :])
            nc.vector.tensor_copy(t[:, :, 0], zero_r[:, :])
            nc.vector.tensor_copy(t[:, :, WP - 1], zero_r[:, :])
            for h in range(2):
                xt = xtmp_p.tile([128, 2048], F32)
                nc.sync.dma_start(
                    out=xt,
                    in_=x_d[it * 128:(it + 1) * 128, h * 2048:(h + 1) * 2048],
                )
                nc.vector.tensor_copy(
                    t[:, 1 + h * 32:1 + h * 32 + 32, 1:1 + W],
                    xt[:, :].rearrange("p (r c) -> p r c", c=W),
                )

        # ---- w: modulate (scale by s_gain) + round to fp32r; q_s[it][i,o] = sum_kk w_mod^2
        w_r = [[None] * (KS * KS) for _ in range(IT)]
        q_s = []
        for it in range(IT):
            q = persist.tile([128, COUT], F32, tag=f"q{it}")
            q_s.append(q)
            for kk in range(KS * KS):
                wt = wtmp_p.tile([128, COUT], F32)
                nc.sync.dma_start(out=wt, in_=w_d[kk, it * 128:(it + 1) * 128, :])
                nc.vector.tensor_scalar_mul(wt[:, :], wt[:, :], s_gain[it][:, :1])
                wr = persist.tile([128, COUT], F32R, tag=f"w{it}_{kk}")
                nc.vector.tensor_copy(wr[:, :], wt[:, :])
                w_r[it][kk] = wr
                wrf = wr[:, :].bitcast(F32)
                if kk == 0:
                    nc.vector.tensor_mul(q[:, :], wrf, wrf)
                else:
                    sq = wsq_p.tile([128, COUT], F32)
                    nc.vector.tensor_mul(sq[:, :], wrf, wrf)
                    nc.vector.tensor_add(q[:, :], q[:, :], sq[:, :])

        # ---- sigma_inv[o] = 1/sqrt(sum_i q + eps): matmuls against ones ----
        sig_ps = sps_p.tile([128, OT], F32)
        for ot in range(OT):
            for it in range(IT):
                nc.tensor.matmul(
                    sig_ps[:, ot:ot + 1],
                    q_s[it][:, ot * 128:(ot + 1) * 128],
                    ones[:, :],
                    start=(it == 0),
                    stop=(it == IT - 1),
                )
        eps_t = persist.tile([128, 1], F32, tag="eps")
        nc.vector.memset(eps_t[:, :], EPS)
        sig_sb = persist.tile([128, OT], F32, tag="sig_sb")
        nc.scalar.activation(
            sig_sb[:, :], sig_ps[:, :], mybir.ActivationFunctionType.Sqrt,
            bias=eps_t[:, :1],
        )
        sinv = persist.tile([128, OT], F32, tag="sinv")
        nc.vector.reciprocal(sinv[:, :], sig_sb[:, :])

        # ---- conv: 36 accumulating matmuls per (pixel tile, out tile) ----
        for nt in range(NT):
            r0 = nt * ROWS_PER_NT
            for ot in range(OT):
                ps = cps_p.tile([128, 512], F32)
                n_mm = IT * KS * KS
                mm = 0
                for it in range(IT):
                    for kk in range(KS * KS):
                        ky, kx = kk // KS, kk % KS
                        rhs = xp[it][:, r0 + ky:r0 + ky + ROWS_PER_NT, kx:kx + W]
                        nc.tensor.matmul(
                            ps[:, :],
                            w_r[it][kk][:, ot * 128:(ot + 1) * 128],
                            rhs,
                            start=(mm == 0),
                            stop=(mm == n_mm - 1),
                        )
                        mm += 1
                ob = out_p.tile([128, 512], F32)
                nc.vector.tensor_scalar_mul(ob[:, :], ps[:, :], sinv[:, ot:ot + 1])
                nc.sync.dma_start(
                    out=y_d[ot * 128:(ot + 1) * 128, nt * 512:(nt + 1) * 512],
                    in_=ob[:, :],
                )

    nc.compile()
    return nc


def _get_nc():
    global _NC_CACHE
    if _NC_CACHE is None:
        _NC_CACHE = _build()
    return _NC_CACHE


def make_in_maps(feature_map, style, weight):
    feature_map = np.ascontiguousarray(np.asarray(feature_map, dtype=np.float32))
    style = np.ascontiguousarray(np.asarray(style, dtype=np.float32))
    weight = np.ascontiguousarray(np.asarray(weight, dtype=np.float32))
    # wt[kk, i, o] = weight[o, i, ky, kx]
    wt = np.ascontiguousarray(weight.transpose(2, 3, 1, 0).reshape(KS * KS, CIN, COUT))
    in_maps = []
    for b in range(B):
        in_maps.append({
            "x": np.ascontiguousarray(feature_map[b].reshape(CIN, PIX)),
            "style4": np.ascontiguousarray(style[b].reshape(IT, 128)),
            "wt": wt,
        })
    return in_maps


def kernel(feature_map, style, weight, _trace=False):
    nc = _get_nc()
    in_maps = make_in_maps(feature_map, style, weight)
    res = run_bass_kernel_spmd(nc, in_maps, core_ids=list(range(B)), trace=_trace)
    y = np.stack([np.asarray(res.results[c]["y"]).reshape(COUT, H, W) for c in range(B)])
    out = y.astype(np.float32)
    if _trace:
        return out, res
    return out
